# revision 9
# baseline (speedup 1.0000x reference)
"""Trainium2 Bass kernel for nn_Binary_CNN2 (binarized CNN, eval mode).

Data-parallel over 8 NeuronCores: batch 4096 -> 512 per core.

v2: fused conv+FC1 phase. The conv maxpool/sign epilogue (DVE+ACT) is the
per-lam bottleneck, so FC1 matmuls are interleaved into the conv phase to
fill PE stalls. FC1 is restructured into 4 ht-groups of 4 PSUM banks each;
each group accumulates its 4 lam-rounds without any flush (PSUM holds the
partial sums), then BN2 + the FC2 matmul for that group run immediately.
FC2 uses W3 as the stationary operand (16 N=512 matmuls into a [10,512]
psum) and a PE transpose to get log-softmax back to batch-major.

Pipeline per core:
  s0:   x [512,1,28,28] f32 -> sign bf16 -> DMA-transpose -> xpad DRAM
  conv: im2col (SWDGE cast to fp8) -> block-diag matmul K=36 -> psum
        epilogue mix per unit: P (DVE pair-max tree + ACT sign),
        Q (ACT sign4 + DVE maxes), S (ACT sign4 + gpsimd maxes)
        -> a [128,4,14,512] fp8 {+-1}
  FC1:  interleaved: z[ht] = sum W2b.T @ a, fp8 DoubleRow, 4 ht-groups
        x 4 lam-rounds accumulating in psum; BN2 affine+clip -> zt bf16
  FC2:  logits.T [10,512] = sum w3[ht].T @ zt[ht] (+b3 via ones row),
        exp/sum/ln (no max-sub needed; |logits| small), PE transpose,
        out = logits - lnS -> [512, 10] f32
"""

import numpy as np
import ml_dtypes

import concourse.bass as bass
import concourse.mybir as mybir
import concourse.tile as tile
from concourse import bacc
from concourse.bass_utils import run_bass_kernel_spmd

EPS = 1e-5
NCORES = 8
B = 512          # batch per core
BH = 256         # batch half (conv matmul free dim)
H = 2048
C = 10
F32 = mybir.dt.float32
BF16 = mybir.dt.bfloat16
FP8 = mybir.dt.float8e4

# conv row-groups over the 28 image rows: sizes 8,8,8,4 (pool-pair aligned)
# valid pooled-row-pair indices per group: g<3 -> ilp 0..3, g=3 -> ilp 0..1
NPART_FOR_ILP = [128, 128, 96, 96]  # FC1 contraction rows valid per ilp

# epilogue pathways (PSUM feeds only one DVE input; Pool runs only TT-add):
#   P: DVE max-reduce + ACT sign           (DVE 1224ns, ACT 507ns)
#   Q: ACT sign4 + DVE bf16 maxes          (ACT 1147ns, DVE 836ns)
#   S: ACT sign4 + gpsimd adds + ACT sign  (ACT 1654ns, GP ~2000ns)
# mix tuned so DVE/ACT/GP all land near ~24us per lam era
EPI_SEQ = ["S" if u in (2, 5, 9, 12, 16, 19, 23, 26) else
           ("Q" if u == 27 else "P") for u in range(28)]


def build_nc(loop_n=None, parts=("s0", "main", "tail"), simplify=None):
    parts = set(parts)
    if parts & {"conv", "fc1"}:
        parts.add("main")
    if "fc2" in parts:
        parts.add("tail")
    nc = bacc.Bacc("TRN2", target_bir_lowering=False, debug=False,
                   num_devices=NCORES)

    xin = nc.dram_tensor("x", [B, 28 * 28], F32, kind="ExternalInput")
    wc = nc.dram_tensor("wc", [36, 128], FP8, kind="ExternalInput")
    negt1 = nc.dram_tensor("negt1", [128, 1], F32, kind="ExternalInput")
    # FC1 weights: [g, lam, k, ht, cp, r, hh]
    w2g = nc.dram_tensor("w2g", [4, 4, 128, 4, 7, 2, 128], FP8,
                         kind="ExternalInput")
    s2t = nc.dram_tensor("s2t", [128, 16], F32, kind="ExternalInput")
    t2t = nc.dram_tensor("t2t", [128, 16], F32, kind="ExternalInput")
    w3t = nc.dram_tensor("w3t", [16, 128, C], BF16, kind="ExternalInput")
    b3r1 = nc.dram_tensor("b3r1", [1, C], BF16, kind="ExternalInput")
    eye10 = nc.dram_tensor("eye10", [C, C], F32, kind="ExternalInput")
    out = nc.dram_tensor("out", [B, C], F32, kind="ExternalOutput")

    # padded transposed image: xpad[i' (34 incl 4 slack), j' (32), b] bf16
    xpad = nc.dram_tensor("xpad", [34 * 32 * B], BF16, kind="Internal")

    hw_q = [nc.sync, nc.scalar]  # two HWDGE issue queues

    with tile.TileContext(nc) as tc:
        with (
            tc.tile_pool(name="consts", bufs=1) as consts,
            tc.tile_pool(name="persist", bufs=1) as persist,
        ):
            # ---- constants to SBUF (outside any timing loop) ----
            wc_sb = consts.tile([36, 128], FP8)
            nc.sync.dma_start(wc_sb[:], wc.ap())
            negt1_sb = consts.tile([128, 1], F32)
            nc.sync.dma_start(negt1_sb[:], negt1.ap())
            s2_sb = consts.tile([128, 16], F32)
            nc.sync.dma_start(s2_sb[:], s2t.ap())
            t2_sb = consts.tile([128, 16], F32)
            nc.sync.dma_start(t2_sb[:], t2t.ap())
            w3_sb = consts.tile([128, 16, C], BF16)
            nc.sync.dma_start(w3_sb[:], w3t.ap().rearrange("t p c -> p t c"))
            b3_sb = consts.tile([1, C], BF16)
            nc.sync.dma_start(b3_sb[:], b3r1.ap())
            eye_sb = consts.tile([C, C], F32)
            nc.sync.dma_start(eye_sb[:], eye10.ap())
            ones10_sb = consts.tile([C, 1], F32)
            nc.vector.memset(ones10_sb[:], 1.0)
            ones1_sb = consts.tile([1, B], BF16)
            nc.vector.memset(ones1_sb[:], 1.0)
            plus3_sb = consts.tile([128, 1], F32)
            nc.vector.memset(plus3_sb[:], 3.0)

            a_sb = persist.tile([128, 4, 14, B], FP8)       # {+-1}
            zt_sb = persist.tile([128, 16, B], BF16)        # clipped BN2 out

            def _body_s0():
              with tc.tile_pool(name="stage0", bufs=1) as s0:
                x_sb = s0.tile([128, 4, 28 * 28], F32, tag="x")
                nc.sync.dma_start(
                    x_sb[:], xin.ap().rearrange("(bo p) f -> p bo f", p=128))
                xb_sb = s0.tile([128, 4, 28, 32], BF16, tag="xb")
                nc.vector.memset(xb_sb[:], 0.0)
                # sign: (x >= 0) - 0.5 -> {+0.5, -0.5}; conv weights carry x2
                nc.vector.tensor_scalar(
                    xb_sb[:, :, :, 0:28],
                    x_sb[:].rearrange("p bo (h w) -> p bo h w", h=28),
                    0.0, 0.5, mybir.AluOpType.is_ge, mybir.AluOpType.subtract)

                # zero the whole xpad buffer (borders stay 0)
                zeros_sb = s0.tile([128, 1088], BF16, tag="zeros")
                nc.vector.memset(zeros_sb[:], 0.0)
                for q in range(4):
                    nc.gpsimd.dma_start(
                        bass.AP(xpad, q * 128 * 1088,
                                [[1088, 128], [1, 1088]]),
                        zeros_sb[:])

                # transpose b <-> (i,j32) in 128x128 tiles, on both HWDGE qs
                xT_sb = s0.tile([128, 7, 4, 128], BF16, tag="xT")
                for c in range(7):
                    for bo in range(4):
                        src = xb_sb[:, bo].rearrange("p h w -> p (h w)")
                        hw_q[(c * 4 + bo) % 2].dma_start(
                            xT_sb[:, c, bo, :],
                            src[:, c * 128:(c + 1) * 128],
                            transpose=True)
                # write interior of xpad at element offset 33*512
                # dst(q,c,bo,bl) = (c*128+q)*512 + 33*512 + bo*128 + bl
                nc.gpsimd.dma_start(
                    bass.AP(xpad, 33 * B,
                            [[B, 128], [128 * B, 7], [128, 4], [1, 128]]),
                    xT_sb[:])

            def _emit_epilogue(kind, ptmp, psq, a_slice):
                if kind == "P":
                    # DVE strided max-reduce (psum can only feed one DVE
                    # input, so no pair-max TT tree here), ACT sign
                    pm = ptmp.tile([128, BH], BF16, tag="pm")
                    nc.vector.tensor_reduce(
                        pm[:],
                        psq[:].rearrange("p s b -> p b s"),
                        axis=mybir.AxisListType.X,
                        op=mybir.AluOpType.max)
                    nc.scalar.activation(
                        a_slice, pm[:],
                        mybir.ActivationFunctionType.Sign,
                        bias=negt1_sb[:])
                else:  # Q/S: sign-first on ACT, then combine the 4 slots
                    sq = ptmp.tile([128, 4, BH], BF16, tag="sq")
                    nc.scalar.activation(
                        sq[:], psq[:],
                        mybir.ActivationFunctionType.Sign,
                        bias=negt1_sb[:])
                    m1 = ptmp.tile([128, 2, BH], BF16, tag="m1")
                    if kind == "Q":
                        nc.vector.tensor_tensor(
                            m1[:], sq[:, 0:2, :], sq[:, 2:4, :],
                            mybir.AluOpType.max)
                        nc.vector.tensor_tensor(
                            a_slice, m1[:, 0, :], m1[:, 1, :],
                            mybir.AluOpType.max)
                    else:
                        nc.gpsimd.tensor_tensor(
                            m1[:], sq[:, 0:2, :], sq[:, 2:4, :],
                            mybir.AluOpType.add)
                        ssum = ptmp.tile([128, BH], BF16, tag="ssum")
                        nc.gpsimd.tensor_tensor(
                            ssum[:], m1[:, 0, :], m1[:, 1, :],
                            mybir.AluOpType.add)
                        nc.scalar.activation(
                            a_slice, ssum[:],
                            mybir.ActivationFunctionType.Sign,
                            bias=plus3_sb[:])

            def _body_main():
              with (
                  tc.tile_pool(name="ptmp", bufs=6) as ptmp,
                  tc.tile_pool(name="w2pool", bufs=2) as w2p,
                  tc.tile_pool(name="zpsum", bufs=1, space="PSUM") as zps,
              ):
                psz = zps.tile([128, 4, B], F32)  # 4 banks: one ht-group

                def _load_w2(g, lam):
                    w2_sb = w2p.tile([128, 4, 7, 2, 128], FP8, tag="w2")
                    nc.sync.dma_start(w2_sb[:], w2g.ap()[g][lam])
                    return w2_sb

                def _f_mm(w2_sb, g, lam, ht, cp, half=None):
                    kk = NPART_FOR_ILP[lam]
                    jp = 2 * cp
                    bs = slice(None) if half is None else (
                        slice(0, BH) if half == 0 else slice(BH, B))
                    nc.tensor.matmul(
                        psz[:, ht, bs],
                        w2_sb[0:kk, ht, cp, :, :],
                        a_sb[0:kk, lam, jp:jp + 2, bs],
                        start=(lam == 0 and cp == 0),
                        stop=(lam == 3 and cp == 6),
                        perf_mode=mybir.MatmulPerfMode.DoubleRow,
                        skip_group_check=True)

                def _bn2_fc2(g, ht, lps_psl):
                    htg = 4 * g + ht
                    nc.scalar.activation(
                        zt_sb[:, htg, :], psz[:, ht, :],
                        mybir.ActivationFunctionType.Identity,
                        bias=t2_sb[:, htg:htg + 1],
                        scale=s2_sb[:, htg:htg + 1])
                    nc.vector.tensor_scalar(
                        zt_sb[:, htg, :], zt_sb[:, htg, :],
                        1.0, -1.0, mybir.AluOpType.min, mybir.AluOpType.max)
                    nc.tensor.matmul(
                        lps_psl[:],
                        w3_sb[:, htg, :],
                        zt_sb[:, htg, :],
                        start=(htg == 0), stop=False,
                        skip_group_check=True)

                with (
                    tc.tile_pool(name="im2col", bufs=2) as imp,
                    tc.tile_pool(name="cpsum", bufs=2, space="PSUM") as cps,
                ):
                  for lam in range(4):          # pooled-row-pair index (ilp)
                    rhs_t = imp.tile([36, 2, 28, B], FP8, tag="rhs")
                    # one SWDGE cast-DMA (bf16->fp8) per (dy,dx):
                    # [4 g-rows, 2 r-rows, 28*512 contiguous (j,b)]
                    for dy in range(3):
                        for dx in range(3):
                            p0 = dx * 12 + dy * 4
                            off = (2 * lam + dy) * 32 * B + dx * B
                            srcap = bass.AP(
                                xpad, off,
                                [[8 * 32 * B, 4], [32 * B, 2],
                                 [1, 28 * B]])
                            nc.gpsimd.dma_start(rhs_t[p0:p0 + 4], srcap)
                    # FC1 fill round for this conv round: F(0, lam-1)
                    if lam >= 1:
                        w2_sb = _load_w2(0, lam - 1)
                        fills = [(w2_sb, 0, lam - 1, ht, cp)
                                 for ht in range(4) for cp in range(7)]
                    else:
                        fills = []
                    nfill = len(fills)
                    for u in range(28):
                        if lam == 3:
                            # jp-major: finish whole jp-pairs early so the
                            # post-conv F(0,3) round can chase the epilogue
                            jp, bh = divmod(u, 2)
                        else:
                            bh, jp = divmod(u, 14)
                        fills_due = (nfill * (u + 1)) // 28 \
                            - (nfill * u) // 28
                        # slot = s*2 + r: each pool pair spans both banks
                        psq = cps.tile([128, 4, BH], F32, tag="cq")
                        for r in range(2):
                            for s in range(2):
                                nc.tensor.matmul(
                                    psq[:, s * 2 + r, :],
                                    wc_sb[:],
                                    rhs_t[:, r, 2 * jp + s,
                                          bh * BH:(bh + 1) * BH],
                                    start=True, stop=True)
                        for _ in range(min(fills_due, len(fills))):
                            _f_mm(*fills.pop(0))
                        a_slice = a_sb[:, lam, jp, bh * BH:(bh + 1) * BH]
                        kind = EPI_SEQ[u]
                        _emit_epilogue(kind, ptmp, psq, a_slice)
                # conv pools closed: 4 psum banks free for FC2
                with tc.tile_pool(name="lpsum", bufs=1, space="PSUM") as lps:
                    psl = lps.tile([C, B], F32, tag="l")
                    # remaining FC1 rounds: F(0,3), then groups 1..3
                    rounds = [(0, 3)] + [
                        (g, lam) for g in range(1, 4) for lam in range(4)]
                    for g, lam in rounds:
                        w2_sb = _load_w2(g, lam)
                        if g == 0:
                            # cp-outer: delays the chunks that depend on the
                            # still-draining lam3 epilogue to the round's end
                            for cp in range(7):
                                for ht in range(4):
                                    _f_mm(w2_sb, g, lam, ht, cp)
                            for ht in range(4):
                                _bn2_fc2(g, ht, psl)
                            continue
                        for ht in range(4):
                            for cp in range(7):
                                _f_mm(w2_sb, g, lam, ht, cp)
                            if lam == 3:
                                _bn2_fc2(g, ht, psl)
                    # b3 row via ones rhs closes the psl accumulation
                    nc.tensor.matmul(
                        psl[:], b3_sb[:], ones1_sb[:],
                        start=False, stop=True, skip_group_check=True)

                    if "tail" in parts:
                        _body_tail(psl, lps)
                    else:
                        ob = ptmp.tile([128, 4, C], F32, tag="ob")
                        nc.vector.memset(ob[:], 0.0)
                        nc.sync.dma_start(
                            out.ap().rearrange("(bo p) c -> p bo c", p=128),
                            ob[:])

            def _body_tail(psl, lps):
              with tc.tile_pool(name="cctmp", bufs=1) as cct:
                # log-softmax without max-subtraction: |logits| <~ 35 so
                # exp stays in f32 range; lnS absorbs the scale.
                e_sb = cct.tile([C, B], F32, tag="e")
                nc.scalar.activation(
                    e_sb[:], psl[:], mybir.ActivationFunctionType.Exp)
                pss = lps.tile([1, B], F32, tag="ss")
                nc.tensor.matmul(pss[:], ones10_sb[:], e_sb[:],
                                 start=True, stop=True)
                lnS_sb = cct.tile([1, B], F32, tag="lnS")
                nc.scalar.activation(
                    lnS_sb[:], pss[:], mybir.ActivationFunctionType.Ln)
                lg_sb = cct.tile([C, B], F32, tag="lg")
                nc.scalar.copy(lg_sb[:], psl[:])
                psT = lps.tile([128, 4, C], F32, tag="T")
                psn = lps.tile([128, 4, 1], F32, tag="n")
                for bt in range(4):
                    nc.tensor.transpose(
                        psT[:, bt, :], lg_sb[:, bt * 128:(bt + 1) * 128],
                        eye_sb[:])
                    nc.tensor.transpose(
                        psn[:, bt, :], lnS_sb[:, bt * 128:(bt + 1) * 128],
                        eye_sb[0:1, 0:1])
                negl = cct.tile([128, 4, 1], F32, tag="negl")
                nc.vector.tensor_scalar_mul(negl[:], psn[:], -1.0)
                out_sb = cct.tile([128, 4, C], F32, tag="out")
                for bt in range(4):
                    nc.scalar.activation(
                        out_sb[:, bt, :], psT[:, bt, :],
                        mybir.ActivationFunctionType.Identity,
                        bias=negl[:, bt, :])
                nc.sync.dma_start(
                    out.ap().rearrange("(bo p) c -> p bo c", p=128),
                    out_sb[:])

            def body():
                if "s0" in parts:
                    _body_s0()
                if "main" in parts:
                    _body_main()
                else:
                    with tc.tile_pool(name="stub", bufs=1) as stub:
                        ob = stub.tile([128, 4, C], F32)
                        nc.vector.memset(ob[:], 0.0)
                        nc.sync.dma_start(
                            out.ap().rearrange("(bo p) c -> p bo c", p=128),
                            ob[:])

            if loop_n is None:
                body()
            else:
                with tc.For_i(0, loop_n, 1):
                    body()

    nc.finalize()
    return nc


_NC_CACHE = {}


def _get_nc(loop_n=None, parts=("s0", "main", "tail")):
    key = (loop_n, tuple(sorted(parts)))
    if key not in _NC_CACHE:
        _NC_CACHE[key] = build_nc(loop_n, parts)
    return _NC_CACHE[key]


def _f(c, k):
    """FC1 feature index map: chunk c=(ilp*14+jp), row k=(g*32+o) -> flat f."""
    ilp, jp = divmod(c, 14)
    g, o = divmod(k, 32)
    if g < 3:
        ip = 4 * g + ilp
    else:
        if ilp >= 2:
            return None
        ip = 12 + ilp
    return o * 196 + ip * 14 + jp


def _host_prep(W1, b1, g1, be1, m1, v1, W2, b2, g2, be2, m2, v2, W3, b3):
    """Precompute small device-side constant tensors (numpy)."""
    s1 = (g1 / np.sqrt(v1 + EPS)).astype(np.float32)
    assert np.all(s1 != 0)
    # bn1 >= 0  <=>  sign(conv_nb - t1[o]) == sign(s1[o]); fold sign(s1)
    # into W2's columns so the device only computes sign(conv_nb - t1)
    t1 = (m1 - be1 / s1 - b1).astype(np.float32)
    sgn1 = np.where(s1 >= 0, 1.0, -1.0).astype(np.float32)
    negt1 = np.repeat(-t1[None, :], 4, axis=0).reshape(128, 1)

    wc = np.zeros((36, 128), np.float32)
    w1s = np.where(W1[:, 0] >= 0, 2.0, -2.0).astype(np.float32)  # [32,3,3] x2
    for dy in range(3):
        for dx in range(3):
            for g in range(4):
                p = dx * 12 + dy * 4 + g
                wc[p, g * 32:(g + 1) * 32] = w1s[:, dy, dx]
    wc = wc.astype(ml_dtypes.float8_e4m3)

    w2s = np.where(W2 >= 0, 1.0, -1.0).astype(np.float32)  # [H, F1]
    w2s = w2s * sgn1[np.arange(w2s.shape[1]) // 196][None, :]
    w2bp = np.zeros((16, 128, 56, 128), np.float32)  # [ht, k, c, hh]
    for c in range(56):
        ilp, jp = divmod(c, 14)
        for g in range(4):
            if _f(c, g * 32) is None:
                continue
            ip = 4 * g + ilp if g < 3 else 12 + ilp
            fs = np.arange(32) * 196 + ip * 14 + jp  # f for o=0..31
            # w2bp[ht, g*32+o, c, hh] = w2s[ht*128+hh, fs[o]]
            blk = w2s[:, fs].reshape(16, 128, 32)   # [ht, hh, o]
            w2bp[:, g * 32:(g + 1) * 32, c, :] = blk.transpose(0, 2, 1)
    # [HT, k, c=(lam,cp,r), hh] -> [g, lam, k, ht, cp, r, hh]
    w2gp = w2bp.reshape(4, 4, 128, 4, 7, 2, 128)    # [g, ht, k, lam, cp, r, hh]
    w2gp = np.ascontiguousarray(w2gp.transpose(0, 3, 2, 1, 4, 5, 6))
    w2gp = w2gp.astype(ml_dtypes.float8_e4m3)

    s2 = (g2 / np.sqrt(v2 + EPS)).astype(np.float32)
    t2 = (be2 + s2 * (b2 - m2)).astype(np.float32)
    s2t = s2.reshape(16, 128).T.copy()
    t2t = t2.reshape(16, 128).T.copy()

    w3t = np.ascontiguousarray(W3.T).reshape(16, 128, C)
    w3t = w3t.astype(ml_dtypes.bfloat16)
    b3r1 = b3[None, :].astype(ml_dtypes.bfloat16)
    eye = np.eye(C, dtype=np.float32)
    return dict(wc=wc, negt1=negt1, w2g=w2gp, s2t=s2t, t2t=t2t,
                w3t=w3t, b3r1=np.ascontiguousarray(b3r1), eye10=eye)


def _make_in_maps(x, consts):
    xs = np.asarray(x, np.float32).reshape(NCORES, B, 28 * 28)
    in_maps = []
    for i in range(NCORES):
        m = {"x": np.ascontiguousarray(xs[i])}
        m.update(consts)
        in_maps.append(m)
    return in_maps


def _prep_all(inputs):
    names = ["W1", "b1", "g1", "be1", "m1", "v1", "W2", "b2", "g2", "be2",
             "m2", "v2", "W3", "b3"]
    return _host_prep(*[np.asarray(inputs[n], np.float32) for n in names])


def kernel(x, **weights):
    consts = _prep_all(weights)
    nc = _get_nc(None)
    in_maps = _make_in_maps(x, consts)
    res = run_bass_kernel_spmd(nc, in_maps, core_ids=list(range(NCORES)))
    outs = [res.results[i]["out"] for i in range(NCORES)]
    return np.concatenate(outs, axis=0).astype(np.float32)


def _make_runner(nc, in_maps):
    """Build a reusable executor with inputs resident on device (no re-upload)."""
    import jax
    import jax.numpy as jnp
    from jax.sharding import Mesh, PartitionSpec, NamedSharding
    from jax.experimental.shard_map import shard_map
    from concourse import bass2jax
    from concourse.bass2jax import _bass_exec_p, install_neuronx_cc_hook

    install_neuronx_cc_hook()
    n_cores = len(in_maps)
    partition_name = nc.partition_id_tensor.name if nc.partition_id_tensor else None
    in_names, out_names, out_avals, zero_outs = [], [], [], []
    for alloc in nc.m.functions[0].allocations:
        if not isinstance(alloc, mybir.MemoryLocationSet):
            continue
        name = alloc.memorylocations[0].name
        if alloc.kind == "ExternalInput":
            if name != partition_name:
                in_names.append(name)
        elif alloc.kind == "ExternalOutput":
            shape = tuple(alloc.tensor_shape)
            dtype = mybir.dt.np(alloc.dtype)
            out_names.append(name)
            out_avals.append(jax.core.ShapedArray(shape, dtype))
            zero_outs.append(np.zeros(shape, dtype))
    n_params = len(in_names)
    n_outs = len(out_avals)
    in_names.extend(out_names)
    if partition_name is not None:
        in_names.append(partition_name)
    donate = tuple(range(n_params, n_params + n_outs))

    def _body(*args):
        operands = list(args)
        if partition_name is not None:
            operands.append(bass2jax.partition_id_tensor())
        outs = _bass_exec_p.bind(
            *operands, out_avals=tuple(out_avals), in_names=tuple(in_names),
            out_names=tuple(out_names), lowering_input_output_aliases=(),
            sim_require_finite=True, sim_require_nnan=True, nc=nc)
        return tuple(outs)

    devices = jax.devices()[:n_cores]
    mesh = Mesh(np.asarray(devices), ("core",))
    sharded = jax.jit(
        shard_map(_body, mesh=mesh,
                  in_specs=(PartitionSpec("core"),) * (n_params + n_outs),
                  out_specs=(PartitionSpec("core"),) * n_outs,
                  check_rep=False),
        donate_argnums=donate, keep_unused=True)
    shard = NamedSharding(mesh, PartitionSpec("core"))
    per_core = [[np.asarray(m[nm]) for nm in in_names[:n_params]]
                for m in in_maps]
    dev_in = [jax.device_put(
                np.concatenate([per_core[c][i] for c in range(n_cores)],
                               axis=0), shard)
              for i in range(n_params)]
    concat_zero_shapes = [((n_cores * z.shape[0],) + z.shape[1:], z.dtype)
                          for z in zero_outs]

    def run():
        zeros = [jnp.zeros(s, d, device=shard) for s, d in concat_zero_shapes]
        outs = sharded(*dev_in, *zeros)
        jax.block_until_ready(outs)
        return outs

    return run


def measure_exec_ns(inputs, n_lo=4, n_hi=132, reps=11):
    """HW exec time per pipeline iteration via looped-kernel wall-clock delta."""
    import time
    consts = _prep_all(inputs)
    in_maps = _make_in_maps(inputs["x"], consts)

    def med_time(loop_n):
        nc = _get_nc(loop_n, measure_exec_ns.parts)
        run = _make_runner(nc, in_maps)
        run()  # compile + warm
        ts = []
        for _ in range(reps):
            t0 = time.time()
            run()
            ts.append(time.time() - t0)
        ts.sort()
        return ts[len(ts) // 2], ts

    t_lo, all_lo = med_time(n_lo)
    t_hi, all_hi = med_time(n_hi)
    measure_exec_ns.last = (all_lo, all_hi)
    return (t_hi - t_lo) / (n_hi - n_lo) * 1e9


measure_exec_ns.parts = ("s0", "main", "tail")
build_nc_looped = build_nc  # marker for test.py
